# revision 43
# baseline (speedup 1.0000x reference)
# Trainium2 Bass kernel for nn_LocalLayer (banded/local linear layer).
#
#   reference: y = x @ W.T + b
#     x [8192, 4096] f32, W [4096, 4096] f32 (block-banded: 256 windows x 16
#     outputs, window k reads inputs [16k-32, 16k+32) clipped to [0, 4096)),
#     b [4096] f32.
#
# Strategy (8 NeuronCores, data-parallel over batch).  The kernel is HBM
# bound (~358 GB/s per core), so everything is organized around minimum
# bytes moved at max sustained rate:
#   - Precision: single bf16 matmul term (x_bf16 @ W_bf16) accumulated in
#     f32 PSUM, bias added during PSUM evacuation, output stored bf16 and
#     converted to f32 on host.  Measured rel err ~4e-3 vs the 2e-2 gate.
#   - Host: per core, transpose + zero-pad the batch shard by 32 rows (top)
#     to [33*128, 1024] bf16, packed row-block-major into xh [128, 33*1024]
#     so large column-range DMAs are per-partition contiguous.  Band of W
#     gathered into w1 [128, 4096] (dense) and w2 [64, 4096] (only 48 rows
#     of the band spill past each 128-aligned chunk, so chunk 2 runs as a
#     K=64 matmul on a compact stationary).
#   - Device: all input traffic on the Sync HWDGE ring in >=1MB transfers
#     ([w1, xp0, w2, xp1..] so compute starts while w2 is in flight);
#     output DMAs ride behind the x pieces in the ring FIFO (deferring
#     output keeps input at full rate -> the measured stream sustains
#     ~390-430 GB/s vs the ~358 GB/s nominal HBM-per-core limit); the last
#     output groups drain on the Scalar ring so the output tail uses both
#     rings.  A 12-matmul pre-warm burst keeps the PE HAM-warm (2.4 GHz).
#   - Per output tile O (32): 4 bf16 matmuls (K-chunk x batch half) into 2
#     PSUM banks; evacuation split between ScalarE and VectorE, each doing
#     psum + bias -> bf16 out tile.
#   - Host: y = concat of per-core unpacked [1024, 4096] tiles, cast f32.
#
# kernel() is self-contained: shapes/sharding hardcoded, no file reads.

import ml_dtypes
import numpy as np

import concourse.mybir as mybir
import concourse.tile as tile
from concourse import bacc
from concourse.bass_utils import run_bass_kernel_spmd

BF16 = ml_dtypes.bfloat16

BATCH = 8192
IN = 4096
N_CORES = 8
B_CORE = BATCH // N_CORES          # 1024
O_TILES = IN // 128                # 32
PAD_TOP = 32
NBLK = O_TILES + 1                 # 33 x row-blocks of 128 (incl tail pad)
XBLK = 32                          # blocks shipped in xh (block 32 rides w2c)
BC = 512                           # batch chunk (one PSUM bank of f32)
N_BC = B_CORE // BC                # 2
OG = 4                             # output tiles per out-DMA group
OUT_SPLIT = 5                      # first out group routed to the Scalar ring
# x DMA piece sizes in row-blocks (sum = XBLK); all pieces >= 1MB (small
# transfers at the queue head ramp the DMA stream slowly -- measured), and
# each piece sized to land just before the PE needs its first block
X_PIECES = [4, 5, 6, 8, 9]

_NC_CACHE = {}


def _build_nc():
    if "nc" in _NC_CACHE:
        return _NC_CACHE["nc"]
    f32 = mybir.dt.float32
    bf16 = mybir.dt.bfloat16
    nc = bacc.Bacc("TRN2", target_bir_lowering=False, debug=False)
    xh_d = nc.dram_tensor("xh", [128, XBLK * B_CORE], bf16, kind="ExternalInput")
    w1_d = nc.dram_tensor("w1", [128, IN], bf16, kind="ExternalInput")
    # w2 band chunk has only 48 nonzero rows; shipped packed two-column-
    # halves-deep in full 128 partitions and unpacked on device by
    # selection matmuls (full-width DMA keeps all 16 SBUF ports busy).
    # The 32 leftover zero partitions carry x row-block 32 (only 32 real
    # rows: the band tail), unpacked the same way.
    w2c_d = nc.dram_tensor("w2c", [128, IN // 2], bf16, kind="ExternalInput")
    sel_d = nc.dram_tensor("sel", [128, 384], bf16, kind="ExternalInput")
    bias_d = nc.dram_tensor("bias", [128, O_TILES], f32, kind="ExternalInput")
    yt_d = nc.dram_tensor("yt", [128, O_TILES * B_CORE], bf16,
                          kind="ExternalOutput")

    with tile.TileContext(nc) as tc:
        with (
            tc.tile_pool(name="consts", bufs=1) as cpool,
            tc.tile_pool(name="psum", bufs=8, space="PSUM") as ppool,
            tc.tile_pool(name="out", bufs=8) as opool,
        ):
            # PE pre-warm: dummy matmuls on a memset tile keep the HAM
            # activity monitor busy during the DMA ramp so the first real
            # matmuls run at 2.4 GHz instead of the 1.2 GHz cold clock
            warm_t = cpool.tile([128, BC], bf16, name="warm", tag="warm")
            nc.vector.memset(warm_t, 0)
            wps = ppool.tile([128, BC], f32, tag="ps", name="ps_warm")
            for _ in range(8):
                nc.tensor.matmul(wps, warm_t[:, 0:128], warm_t,
                                 start=True, stop=True)

            # inputs on the Sync ring.  Order [w1, w2c, xp0, ...]: the
            # chunk-1 matmuls of the first tiles only need w1 + xp0, so
            # compute starts while the w2 unpack is still in flight
            w1_t = cpool.tile([128, IN], bf16, name="w1", tag="w1")
            w2_t = cpool.tile([128, IN], bf16, name="w2", tag="w2")
            w2c_t = cpool.tile([128, IN // 2], bf16, name="w2c", tag="w2c")
            sel_t = cpool.tile([128, 384], bf16, name="sel", tag="sel")
            nc.sync.dma_start(w1_t, w1_d.ap())
            nc.sync.dma_start(w2c_t, w2c_d.ap())
            bias_t = cpool.tile([128, O_TILES], f32, name="bias", tag="bias")
            nc.scalar.dma_start(sel_t, sel_d.ap())
            nc.scalar.dma_start(bias_t, bias_d.ap())

            # unpack w2: selection matmuls move packed half-columns into
            # band-chunk position (rows 48-127 zeroed by the selector);
            # ACT/DVE evacuate psum -> w2 bf16 while the x stream runs
            for q in range(8):
                half, qq = divmod(q, 4)
                ps_u = ppool.tile([128, BC], f32, tag="ps", name=f"ps_u{q}")
                nc.tensor.matmul(
                    ps_u, sel_t[:, half * 128:(half + 1) * 128],
                    w2c_t[:, qq * BC:(qq + 1) * BC],
                    start=True, stop=True,
                )
                dst = w2_t[:, half * (IN // 2) + qq * BC:
                           half * (IN // 2) + (qq + 1) * BC]
                if q % 2 == 0:
                    nc.scalar.copy(dst, ps_u)
                else:
                    nc.vector.tensor_copy(dst, ps_u)

            # unpack x block 32 from w2c's spare partitions (rows 32-127 of
            # the psum result are zero, matching the tail zero-padding)
            x32_t = cpool.tile([128, B_CORE], bf16, name="x32", tag="x32")
            for bc in range(N_BC):
                ps_x = ppool.tile([128, BC], f32, tag="ps", name=f"ps_x{bc}")
                nc.tensor.matmul(ps_x, sel_t[:, 256:384],
                                 w2c_t[:, bc * BC:(bc + 1) * BC],
                                 start=True, stop=True)
                dst = x32_t[:, bc * BC:(bc + 1) * BC]
                if bc == 0:
                    nc.scalar.copy(dst, ps_x)
                else:
                    nc.vector.tensor_copy(dst, ps_x)

            xp_t = []
            blk_loc = {}       # row-block -> (piece idx, local col offset)
            c0 = 0
            for pi, nb in enumerate(X_PIECES):
                t = cpool.tile([128, nb * B_CORE], bf16, name=f"xp{pi}",
                               tag=f"xp{pi}")
                nc.sync.dma_start(
                    t, xh_d.ap()[:, c0 * B_CORE:(c0 + nb) * B_CORE]
                )
                xp_t.append(t)
                for j in range(nb):
                    blk_loc[c0 + j] = (pi, j * B_CORE)
                c0 += nb

            def xblk(b, bc):
                if b == XBLK:
                    return x32_t[:, bc * BC:(bc + 1) * BC]
                pi, off = blk_loc[b]
                return xp_t[pi][:, off + bc * BC: off + (bc + 1) * BC]

            for g in range(O_TILES // OG):
                og_t = opool.tile([128, OG * B_CORE], bf16, tag="out",
                                  name=f"og{g}")
                for lo in range(OG):
                    O = g * OG + lo
                    osl = slice(O * 128, (O + 1) * 128)
                    pss = [
                        ppool.tile([128, BC], f32, tag="ps", name=f"ps_{O}_{i}")
                        for i in range(N_BC)
                    ]
                    for bc in range(N_BC):
                        nc.tensor.matmul(pss[bc], w1_t[:, osl], xblk(O, bc),
                                         start=True, stop=False)
                    for bc in range(N_BC):
                        nc.tensor.matmul(pss[bc], w2_t[:, osl],
                                         xblk(O + 1, bc),
                                         start=False, stop=True)
                    # evacuation: psum + bias -> bf16, split ACT / DVE
                    ob = lo * B_CORE
                    nc.scalar.add(og_t[:, ob:ob + BC], pss[0],
                                  bias_t[:, O:O + 1])
                    nc.vector.tensor_scalar_add(og_t[:, ob + BC:ob + 2 * BC],
                                                pss[1], bias_t[:, O:O + 1])
                # output DMAs ride behind the x pieces in the Sync ring FIFO
                # (deferred until input streaming is done -> input stays at
                # full rate).  The last few groups -- whose evacuation only
                # completes near the end of the input phase -- go on the
                # Scalar ring so the output tail drains on both rings.
                ring = nc.scalar if g >= OUT_SPLIT else nc.sync
                ring.dma_start(
                    yt_d.ap()[:, g * OG * B_CORE:(g + 1) * OG * B_CORE], og_t
                )

    nc.compile()
    _NC_CACHE["nc"] = nc
    return nc


def _band_gather(W, shift, rows):
    """wc[i, O*128+j] = W[128O+j, 128O+shift+i], zero outside [0, IN)."""
    i = np.arange(rows)[:, None, None]
    O = np.arange(O_TILES)[None, :, None]
    j = np.arange(128)[None, None, :]
    o_idx = np.broadcast_to(128 * O + j, (rows, O_TILES, 128))
    f = 128 * O + shift + i
    wc = np.where(
        (f >= 0) & (f < IN), W[o_idx, np.clip(f, 0, IN - 1)], np.float32(0)
    )
    return wc.reshape(rows, O_TILES * 128)


def kernel(x, W, b, mask=None):
    x = np.asarray(x, dtype=np.float32)
    W = np.asarray(W, dtype=np.float32)

    w1 = _band_gather(W, -PAD_TOP, 128).astype(BF16)
    w2 = _band_gather(W, 128 - PAD_TOP, 48).astype(BF16)
    # pack w2's 48 nonzero rows two-column-halves-deep in 128 partitions;
    # partitions 48-63 / 112-127 carry x block 32's 32 real rows per core
    w2c_base = np.zeros((128, IN // 2), BF16)
    w2c_base[0:48] = w2[:, 0:IN // 2]
    w2c_base[64:112] = w2[:, IN // 2:IN]
    # selection stationaries (out = sel.T @ w2c): cols 0:128 pick w2 half A,
    # 128:256 half B (zeroing rows 48-127), 256:384 rebuild x block 32
    sel = np.zeros((128, 384), BF16)
    sel[np.arange(48), np.arange(48)] = 1
    sel[64 + np.arange(48), 128 + np.arange(48)] = 1
    sel[48 + np.arange(16), 256 + np.arange(16)] = 1
    sel[112 + np.arange(16), 272 + np.arange(16)] = 1
    bias = np.ascontiguousarray(
        np.asarray(b, dtype=np.float32).reshape(O_TILES, 128).T
    )

    xt = x.T  # [4096, 8192] view
    in_maps = []
    for c in range(N_CORES):
        sh = np.zeros((NBLK * 128, B_CORE), np.float32)
        sh[PAD_TOP:PAD_TOP + IN, :] = xt[:, c * B_CORE:(c + 1) * B_CORE]
        shb = sh.astype(BF16)
        xh = np.ascontiguousarray(
            shb[:XBLK * 128].reshape(XBLK, 128, B_CORE).transpose(1, 0, 2)
        ).reshape(128, XBLK * B_CORE)
        w2c = w2c_base.copy()
        w2c[48:64, 0:B_CORE] = shb[XBLK * 128:XBLK * 128 + 16]
        w2c[112:128, 0:B_CORE] = shb[XBLK * 128 + 16:XBLK * 128 + 32]
        in_maps.append(
            {"xh": xh, "w1": w1, "w2c": w2c, "sel": sel, "bias": bias}
        )

    nc = _build_nc()
    res = run_bass_kernel_spmd(nc, in_maps, core_ids=list(range(N_CORES)))
    outs = []
    for r in res.results:
        yt = np.asarray(r["yt"]).reshape(128, O_TILES, B_CORE)
        outs.append(
            yt.transpose(2, 1, 0).reshape(B_CORE, IN).astype(np.float32)
        )
    return np.ascontiguousarray(np.concatenate(outs, axis=0))


if __name__ == "__main__":
    rng = np.random.default_rng(0)
    x = rng.standard_normal((BATCH, IN), dtype=np.float32)
    W = rng.standard_normal((IN, IN), dtype=np.float32)
    b = rng.standard_normal(IN, dtype=np.float32)
    y = kernel(x, W, b)
    print(y.shape, y.dtype)
